# revision 20
# baseline (speedup 1.0000x reference)
"""AdaptiveGraphLearning forward on 8 Trainium2 NeuronCores.

Data-parallel over batch B=64: each core processes 8 batches; the (N,N)
adjacency parameter and tiny edge-MLP weights are replicated (the forward
pass needs no collectives).

v3 dataflow (per core, 8 batches as 4 pairs). v1 was TensorE-bound (87%
busy: X = Wi.T@node_i + Wj.T@node_j as 512-wide broadcast-moving
matmuls). v3 computes tiny projections pi/pj once per pair (256 streamed
columns instead of 8192) and materializes X = relu(pi_i + pj_j) on the
vector engines, leaving PE only the irreducible W2/W3 streams:

  - DMA all pairs as 2MB h-quarter tiles on the SP + ACT HWDGE rings
    (sustains ~430 GB/s; quarters keep the t-fold within ~2us of data
    landing). Consts ride ahead as 2 packed transfers.
  - Sum over t: f32->bf16 fold then bf16 folds (2x DVE mode; all
    operands SBUF, unit stride). Quarters split DVE/Pool per ENG_QUARTER
    (Pool's software ALU is ~2.1 ns/el, so it gets a bounded share; its
    chain is pure tensor_tensor folds to t=1 since Pool lacks axis-X
    reduce).
  - R (128=(b_lo,n), 128=h) bf16 -> node_T via DMA XBAR transpose.
  - pi/pj: 2 matmuls of 128 cols; ACT evacuates (b1 folded into pi).
  - Per chunk (8 i x 64 j, both batches = 1024 cols): X-add on DVE/Pool
    (broadcast APs; stride-0 innermost keeps this 1x), relu on ACT
    (SBUF->SBUF), PE W2 (2x512 cols) -> h2 relu evac (+b2) on ACT/Pool
    -> PE W3 one-hot accumulate into per-batch (8,512) PSUM.
  - Epilogue per batch: F(8,512) -> (64,64) via SBUF DMA reshape, F^T on
    PE, out = (relu(G + F + F^T) + I) / row-sum with G = 0.25*(ap+ap^T)
    precomputed host-side (0.25 sym factor folded into W3/b3).

Harness notes: walrus in this container accepts a single semaphore wait
per instruction, so a BIR-level pass splits Tile's multi-wait
instructions onto EventSemaphore carriers; the Tile kernel-tail drain
gets the same treatment at build time.
"""
import sys

sys.path.insert(0, '/opt/trn_rl_repo')

import numpy as np

B, N, H, T = 64, 64, 128, 128
NCORES = 8
B_LOC = B // NCORES      # 8 batches per core
PAIRS = B_LOC // 2       # 4 batch pairs per core
NCH = N // 8             # 8 i-chunks per batch (8 i x 64 j = 512 wide)
HQ = H // 4              # quarter h-range per DMA tile

# packed const layouts (columns)
CB_WI, CB_WJ, CB_W2, CB_W3 = 0, 128, 256, 320   # bf16 block, width 384
CF_B1, CF_B2, CF_B3, CF_G, CF_I64, CF_Z = 0, 1, 2, 3, 67, 131  # f32, w 132

# --- engine assignment knobs (tuned from traces) ---
# fold engine per quarter index: 'v' = DVE chain, 'g1' = Pool does the
# big f32 fold1, DVE the bf16 tail (Pool chains are slow; one f32 fold1
# per pair balances it)
ENG_QUARTER = ('v', 'v', 'v', 'g1')
# X-add engine per chunk
XADD_ENG = ('v', 'v', 'g', 'g', 'g', 'g', 'g', 'g')
# h2 evacuation engine per chunk ('a' = ACT; 'v' = DVE tensor_scalar)
H2_ENG = ('a', 'a', 'a', 'a', 'a', 'a', 'a', 'a')
# ff evacuation: 'v' = DVE tensor_scalar (PSUM-capable), 'a' = ACT
FF_ENG = 'v'

_CACHE = {}


def _install_wait_splitter():
    """walrus's per-instruction sync structs hold a single semaphore wait;
    Tile can emit several on one instruction. Split extras onto preceding
    single-wait EventSemaphore instructions at the BIR-JSON level."""
    if _CACHE.get('wait_splitter'):
        return
    import json

    import concourse.bass2jax as bass2jax

    orig = bass2jax.compile_bir_kernel

    def split_waits_in_bir(bir_bytes):
        d = json.loads(bir_bytes)
        n_new = [0]
        for fn in d.get("functions", []):
            for bb in fn.get("blocks", []):
                out = []
                for ins in bb.get("instructions", []):
                    si = ins.get("sync_info") or {}
                    waits = si.get("on_wait") or []
                    if len(waits) > 1:
                        for w in waits[:-1]:
                            n_new[0] += 1
                            out.append({
                                "engine": ins["engine"],
                                "ins": [],
                                "name": f"IWS-{n_new[0]}",
                                "opcode": "EventSemaphore",
                                "outs": [],
                                "sync_info": {"on_update": [], "on_wait": [w]},
                            })
                        si["on_wait"] = [waits[-1]]
                    out.append(ins)
                bb["instructions"] = out
        return json.dumps(d).encode()

    def wrapper(ant_bir_str, *a, **kw):
        return orig(split_waits_in_bir(ant_bir_str), *a, **kw)

    bass2jax.compile_bir_kernel = wrapper
    _CACHE['wait_splitter'] = True


def _split_drain_tile_context(tile_mod, mybir, nc):
    """TileContext whose kernel-tail drain splits its semaphore waits across
    sequential Drain instructions (walrus CTRL insts accept one wait)."""
    from concourse.tile import ScopedClock

    class SplitDrainTileContext(tile_mod.TileContext):
        def _drain_and_barrier(self, tick_clock, wait_clock):
            drain_inst = self.nc.sync.drain()
            wait_clock.add_sem_waits(
                drain_inst.ins, ScopedClock({None: tick_clock.global_clock})
            )
            waits = list(drain_inst.ins.sync_info.on_wait)
            if len(waits) > 1:
                drain_inst.ins.sync_info = mybir.SyncInfo(
                    on_wait=waits[:1],
                    on_update=list(drain_inst.ins.sync_info.on_update),
                )
                for i in range(1, len(waits)):
                    extra = self.nc.sync.drain()
                    extra.ins.sync_info = mybir.SyncInfo(
                        on_wait=waits[i : i + 1], on_update=[]
                    )
            self.nc.all_engine_barrier()
            assert self.sems is not None
            popped = self.nc._tile_sem_poison_stack.pop()
            assert popped is self._sem_poison
            self.nc.clear_and_free_semaphores(list(self.sems.allocated().values()))
            self.nc.all_engine_barrier()

    return SplitDrainTileContext(nc)


def build_nc():
    import concourse.bass as bass
    import concourse.tile as tile
    from concourse import mybir
    from contextlib import ExitStack

    f32 = mybir.dt.float32
    bf16 = mybir.dt.bfloat16
    AF = mybir.ActivationFunctionType
    ALU = mybir.AluOpType
    AX = mybir.AxisListType

    nc = bass.Bass()
    tf = nc.declare_dram_parameter("tf", [B_LOC, N, H, T], f32, isOutput=False)
    CB = nc.declare_dram_parameter("CB", [128, 384], bf16, isOutput=False)
    CF = nc.declare_dram_parameter("CF", [128, 132], f32, isOutput=False)
    out_ext = nc.declare_dram_parameter("out", [B_LOC, N, N], f32, isOutput=True)

    NOBIAS = _CACHE.get('cfg_nobias', False)

    with _split_drain_tile_context(tile, mybir, nc) as tc, ExitStack() as ctx, \
            nc.allow_low_precision("bf16 t-fold accumulation within 2e-2 tol"):
        consts = ctx.enter_context(tc.tile_pool(name="consts", bufs=1))
        tf_pool = ctx.enter_context(tc.tile_pool(name="tf", bufs=6))
        fold_pool = ctx.enter_context(tc.tile_pool(name="fold", bufs=2))
        red_pool = ctx.enter_context(tc.tile_pool(name="red", bufs=2))
        pp_pool = ctx.enter_context(tc.tile_pool(name="pp", bufs=2))
        x_pool = ctx.enter_context(tc.tile_pool(name="x", bufs=4))
        h2_pool = ctx.enter_context(tc.tile_pool(name="h2", bufs=3))
        ff_pool = ctx.enter_context(tc.tile_pool(name="ff", bufs=2))
        ep_pool = ctx.enter_context(tc.tile_pool(name="ep", bufs=2))
        ps_h2 = ctx.enter_context(tc.tile_pool(name="ps_h2", bufs=2, space="PSUM"))
        ps_w3 = ctx.enter_context(tc.tile_pool(name="ps_w3", bufs=1, space="PSUM"))
        ps_pp = ctx.enter_context(tc.tile_pool(name="ps_pp", bufs=1, space="PSUM"))
        ps_ft = ctx.enter_context(tc.tile_pool(name="ps_ft", bufs=1, space="PSUM"))

        def load_quarters(c):
            # One pair (2 batches) as 4 x 2MB h-quarter tiles; even
            # quarters ride SP, odd ACT -> both rings stay fed.
            qs = []
            for q in range(4):
                tft = tf_pool.tile([128, HQ, T], f32, name=f"tf{c}_{q}",
                                   tag="tft")
                eng = nc.sync if q % 2 == 0 else nc.scalar
                eng.dma_start(
                    tft[:], tf[2 * c : 2 * c + 2, :, q * HQ : (q + 1) * HQ, :])
                qs.append(tft[:])
            return qs

        # tf streaming starts immediately; consts follow on the ACT ring
        # (small, land within ~1us, needed only ~15us in).
        pending = load_quarters(0)
        cb_sb = consts.tile([128, 384], bf16)
        nc.scalar.dma_start(cb_sb[:], CB[:])
        cf_sb = consts.tile([128, 132], f32)
        nc.scalar.dma_start(cf_sb[:], CF[:])

        wi_sb = cb_sb[:, CB_WI : CB_WI + 128]
        wj_sb = cb_sb[:, CB_WJ : CB_WJ + 128]
        w2_sb = cb_sb[:, CB_W2 : CB_W2 + 64]
        w3_sb = cb_sb[0:64, CB_W3 : CB_W3 + 64]
        b1_sb = cf_sb[:, CF_B1 : CF_B1 + 1]
        b2_sb = cf_sb[0:64, CF_B2 : CF_B2 + 1]
        b3_sb = cf_sb[0:8, CF_B3 : CF_B3 + 1]
        g_sb = cf_sb[0:64, CF_G : CF_G + 64]
        i64_sb = cf_sb[0:64, CF_I64 : CF_I64 + 64]
        zro_sb = cf_sb[0:64, CF_Z : CF_Z + 1]

        def emit_f1(mode, th, c, q):
            # First fold (t 128->64), f32 in, bf16 out (1x). mode 'g1'
            # runs it on Pool to offload DVE.
            eng1 = nc.gpsimd if mode == 'g1' else nc.vector
            f1 = fold_pool.tile([128, HQ, 64], bf16, tag=f"f1{ENG_QUARTER[q]}{q}",
                                name=f"f1_{c}_{q}")
            eng1.tensor_tensor(f1[:], th[:, :, 0:64], th[:, :, 64:128],
                               op=ALU.add)
            return f1

        def emit_fold_tail(f1, r_sb, c, q):
            # bf16 folds (DVE 2x/4x mode) + reduce: t 64 -> 1.
            f2 = fold_pool.tile([128, HQ, 32], bf16, tag=f"f2{q}",
                                name=f"f2_{c}_{q}")
            nc.vector.tensor_tensor(f2[:], f1[:, :, 0:32], f1[:, :, 32:64],
                                    op=ALU.add)
            f3 = fold_pool.tile([128, HQ, 16], bf16, tag=f"f3{q}",
                                name=f"f3_{c}_{q}")
            nc.vector.tensor_tensor(f3[:], f2[:, :, 0:16], f2[:, :, 16:32],
                                    op=ALU.add)
            nc.vector.reduce_sum(r_sb[:, q * HQ : (q + 1) * HQ], f3[:],
                                 axis=AX.X)

        def emit_folds_dve(c, parts, r_sb):
            # DVE-side fold chains (quarters 0..2), in landing order.
            for q in range(4):
                if ENG_QUARTER[q] != 'g1':
                    f1 = emit_f1('v', parts[q], c, q)
                    emit_fold_tail(f1, r_sb, c, q)

        def emit_folds_pool(c, parts):
            # Pool's f32 fold1 for its quarters; returns tail work.
            tails = []
            for q in range(4):
                if ENG_QUARTER[q] == 'g1':
                    tails.append((emit_f1('g1', parts[q], c, q), q))
            return tails

        # Software-pipelined schedule: pair c+1's folds are interleaved
        # into pair c's chunk phase so every engine FIFO is ordered by
        # input-readiness (avoids head-of-line blocking).
        r_all = [red_pool.tile([128, H], bf16, tag="r", name=f"r{c}")
                 for c in range(PAIRS)]
        parts0 = pending
        pending = load_quarters(1)
        emit_folds_dve(0, parts0, r_all[0])
        tails = emit_folds_pool(0, parts0)
        for f1, q in tails:
            emit_fold_tail(f1, r_all[0], 0, q)

        for c in range(PAIRS):
            r_sb = r_all[c]
            # node_T[h, (b_lo, n)] via DMA XBAR transpose (bf16)
            rt_sb = red_pool.tile([H, 128], bf16, tag="rt", name=f"rt{c}")
            nc.sync.dma_start_transpose(rt_sb[:], r_sb[:])

            # pi = Wi.T @ node_T (+b1 on evac), pj = Wj.T @ node_T
            pp_ps = ps_pp.tile([128, 256], f32, tag="pp")
            nc.tensor.matmul(pp_ps[:, 0:128], wi_sb, rt_sb[:],
                             start=True, stop=True)
            nc.tensor.matmul(pp_ps[:, 128:256], wj_sb, rt_sb[:],
                             start=True, stop=True)
            pi_sb = pp_pool.tile([H, 2, 64], bf16, tag="pi", name=f"pi{c}")
            if NOBIAS:
                nc.scalar.activation(pi_sb[:], pp_ps[:, 0:128], AF.Copy)
            else:
                nc.scalar.activation(pi_sb[:], pp_ps[:, 0:128], AF.Identity,
                                     bias=b1_sb)
            pj_sb = pp_pool.tile([H, 2, 64], bf16, tag="pj", name=f"pj{c}")
            nc.scalar.activation(pj_sb[:], pp_ps[:, 128:256], AF.Copy)

            next_parts = pending
            if c + 2 < PAIRS:
                pending = load_quarters(c + 2)

            w3_ps = [ps_w3.tile([8, 512], f32, tag=f"w3_{b}",
                                name=f"w3_{c}_{b}") for b in range(2)]

            def emit_chunk(c2):
                # X_pre = pi[:, b, i] + pj[:, b, j] for chunk's 8 i values,
                # both batches: (128, 2, 8, 64) in one op
                xa = x_pool.tile([H, 2, 8, 64], bf16, tag="xa",
                                 name=f"xa_{c}_{c2}")
                xeng = nc.vector if XADD_ENG[c2] == 'v' else nc.gpsimd
                xeng.tensor_tensor(
                    xa[:],
                    pi_sb[:, :, 8 * c2 : 8 * c2 + 8].unsqueeze(3)
                        .broadcast_to((H, 2, 8, 64)),
                    pj_sb[:].unsqueeze(2).broadcast_to((H, 2, 8, 64)),
                    op=ALU.add)
                x_sb = x_pool.tile([H, 2, 8, 64], bf16, tag="x",
                                   name=f"x_{c}_{c2}")
                nc.scalar.activation(x_sb[:], xa[:], AF.Relu)
                h2_ps = ps_h2.tile([64, 1024], f32, tag="h2ps")
                nc.tensor.matmul(h2_ps[:, 0:512], w2_sb, x_sb[:, 0],
                                 start=True, stop=True)
                nc.tensor.matmul(h2_ps[:, 512:1024], w2_sb, x_sb[:, 1],
                                 start=True, stop=True)
                h2_sb = h2_pool.tile([64, 1024], bf16, tag="h2",
                                     name=f"h2_{c}_{c2}")
                if H2_ENG[c2] == 'v':
                    # relu(x + b2) == max(x, -b2) + b2; with zero bias a
                    # single max suffices
                    if NOBIAS:
                        nc.vector.tensor_scalar(
                            h2_sb[:], h2_ps[:], scalar1=0.0, scalar2=None,
                            op0=ALU.max)
                    else:
                        nc.vector.tensor_scalar(
                            h2_sb[:], h2_ps[:], scalar1=b2_sb, scalar2=0.0,
                            op0=ALU.add, op1=ALU.max)
                else:
                    nc.scalar.activation(h2_sb[:], h2_ps[:], AF.Relu,
                                         bias=0.0 if NOBIAS else b2_sb)
                for b in range(2):
                    nc.tensor.matmul(
                        w3_ps[b][:],
                        w3_sb[:, 8 * c2 : 8 * c2 + 8],
                        h2_sb[:, 512 * b : 512 * b + 512],
                        start=(c2 == 0), stop=(c2 == NCH - 1))

            # chunks with DVE X-adds first, then next pair's DVE folds
            # (their quarters land mid-phase), then Pool-X-add chunks,
            # then Pool's next-pair fold1 — each engine FIFO stays in
            # input-readiness order.
            for c2 in range(NCH):
                if XADD_ENG[c2] == 'v':
                    emit_chunk(c2)
            if c + 1 < PAIRS:
                emit_folds_dve(c + 1, next_parts, r_all[c + 1])
            for c2 in range(NCH):
                if XADD_ENG[c2] != 'v':
                    emit_chunk(c2)
            tails = emit_folds_pool(c + 1, next_parts) \
                if c + 1 < PAIRS else []

            # Epilogue, both batches fused as (64, 2, 64) tiles:
            # out = (relu(G + F + F^T) + I) / (rowsum + 1e-8)
            f_sb = ep_pool.tile([N, 2, N], f32, tag="f")
            ft_ps = ps_ft.tile([N, 2, N], f32, tag="ft")
            for b_lo in range(2):
                # F_flat -> F (64, 64): same linearized element order
                ff_sb = ff_pool.tile([8, 512], f32, tag="ff",
                                     name=f"ff_{c}_{b_lo}")
                if FF_ENG == 'v':
                    nc.vector.tensor_scalar(
                        ff_sb[:], w3_ps[b_lo][:],
                        scalar1=0.0 if NOBIAS else b3_sb, scalar2=None,
                        op0=ALU.add)
                elif NOBIAS:
                    nc.scalar.activation(ff_sb[:], w3_ps[b_lo][:], AF.Copy)
                else:
                    nc.scalar.activation(ff_sb[:], w3_ps[b_lo][:], AF.Identity,
                                         bias=b3_sb)
                nc.sync.dma_start(f_sb[:, b_lo], ff_sb[:])
                nc.tensor.transpose(ft_ps[:, b_lo], f_sb[:, b_lo], i64_sb)
            t1 = ep_pool.tile([N, 2, N], f32, tag="t1")
            nc.vector.tensor_tensor(t1[:], f_sb[:], ft_ps[:], op=ALU.add)
            t2 = ep_pool.tile([N, 2, N], f32, tag="t2")
            nc.vector.tensor_tensor(
                t2[:], t1[:],
                g_sb.unsqueeze(1).broadcast_to((N, 2, N)), op=ALU.add)
            sp = ep_pool.tile([N, 2, N], f32, tag="sp")
            nc.vector.tensor_scalar(
                sp[:], t2[:], scalar1=0.0, scalar2=None, op0=ALU.max)
            spi = ep_pool.tile([N, 2, N], f32, tag="spi")
            nc.vector.tensor_tensor(
                spi[:], sp[:],
                i64_sb.unsqueeze(1).broadcast_to((N, 2, N)), op=ALU.add)
            rs = ep_pool.tile([N, 2], f32, tag="rs")
            nc.vector.reduce_sum(rs[:], spi[:], axis=AX.X)
            rb = ep_pool.tile([N, 2], f32, tag="rb")
            nc.vector.tensor_scalar(
                rb[:], rs[:], scalar1=1e-8, scalar2=None, op0=ALU.add)
            rec = ep_pool.tile([N, 2], f32, tag="rec")
            nc.vector.reciprocal(rec[:], rb[:])
            for b_lo in range(2):
                o_sb = ep_pool.tile([N, N], f32, tag=f"o{b_lo}",
                                    name=f"o_{c}_{b_lo}")
                nc.vector.tensor_scalar(
                    o_sb[:], spi[:, b_lo], scalar1=rec[:, b_lo : b_lo + 1],
                    scalar2=None, op0=ALU.mult)
                nc.sync.dma_start(out_ext[2 * c + b_lo], o_sb[:])

            for f1, q in tails:
                emit_fold_tail(f1, r_all[c + 1], c + 1, q)
    return nc


def _get_nc():
    key = ('nc', _CACHE.get('cfg_nobias', False))
    if key not in _CACHE:
        _CACHE[key] = build_nc()
    return _CACHE[key]


def kernel(**inputs):
    import ml_dtypes

    from concourse.bass_utils import run_bass_kernel_spmd

    _install_wait_splitter()

    tf = np.asarray(inputs["temporal_features"], dtype=np.float32)
    ap = np.asarray(inputs["adj_param"], dtype=np.float32)
    W1 = np.asarray(inputs["W1"], dtype=np.float32)
    b1 = np.asarray(inputs["b1"], dtype=np.float32)
    W2 = np.asarray(inputs["W2"], dtype=np.float32)
    b2 = np.asarray(inputs["b2"], dtype=np.float32)
    W3 = np.asarray(inputs["W3"], dtype=np.float32)
    b3 = np.asarray(inputs["b3"], dtype=np.float32)

    bf = ml_dtypes.bfloat16
    # Per chunk, an (H//2, 8) one-hot-column weight routing the chunk's
    # scalar output to PSUM partition `chunk` (0.25 sym factor folded in).
    W3blk = np.zeros((H // 2, NCH, 8), np.float32)
    for chunk in range(NCH):
        W3blk[:, chunk, chunk] = 0.25 * W3[:, 0]

    CBnp = np.zeros((128, 384), np.float32)
    CBnp[:, CB_WI : CB_WI + 128] = W1[:H] / T
    CBnp[:, CB_WJ : CB_WJ + 128] = W1[H:] / T
    CBnp[:, CB_W2 : CB_W2 + 64] = W2
    CBnp[0:64, CB_W3 : CB_W3 + 64] = W3blk.reshape(H // 2, 8 * NCH)
    CBnp = np.ascontiguousarray(CBnp.astype(bf))

    CFnp = np.zeros((128, 132), np.float32)
    CFnp[:, CF_B1] = b1
    CFnp[0:64, CF_B2] = b2
    CFnp[0:8, CF_B3] = 0.25 * float(b3[0])
    CFnp[0:64, CF_G : CF_G + 64] = 0.25 * (ap + ap.T)
    CFnp[0:64, CF_I64 : CF_I64 + 64] = np.eye(N, dtype=np.float32)
    CFnp = np.ascontiguousarray(CFnp)

    shared = {"CB": CBnp, "CF": CFnp}
    in_maps = [
        {"tf": np.ascontiguousarray(tf[i * B_LOC : (i + 1) * B_LOC]), **shared}
        for i in range(NCORES)
    ]

    _CACHE['cfg_nobias'] = bool(
        not b1.any() and not b2.any() and not b3.any())
    nc = _get_nc()
    res = run_bass_kernel_spmd(nc, in_maps, core_ids=list(range(NCORES)),
                               **_CACHE.get('run_kwargs', {}))
    _CACHE['last_result'] = res
    out = np.concatenate([res.results[i]["out"] for i in range(NCORES)], axis=0)
    return np.ascontiguousarray(out.astype(np.float32))


# revision 21
# speedup vs baseline: 1.2291x; 1.2291x over previous
"""AdaptiveGraphLearning forward on 8 Trainium2 NeuronCores.

Data-parallel over batch B=64: each core processes 8 batches; the (N,N)
adjacency parameter and tiny edge-MLP weights are replicated (the forward
pass needs no collectives).

Per-core dataflow (8 batches as 4 pairs):
  - HWDGE-DMA two batches of temporal_features per transfer as
    (128, 8192) f32 tiles: partition p=(b_lo,n), free=(h,t) -- 32-64KB
    contiguous DRAM per partition for near-peak HBM bandwidth. Pair 0 is
    split into 2MB quarters spread over both HWDGE rings (SP + ACT) to
    shorten the pipeline-fill latency; later pairs ride the SP ring as
    4MB halves while ACT is busy with evacuations.
  - Sum over t: one f32->bf16 VectorE fold, then bf16 folds in 2x mode
    (t:128->64->32->16) and a short 1x reduce -> R (128=(b_lo,n), 128=h).
  - TensorE transpose -> node_T bf16 (h on partitions). 1/T folded into
    the W1 halves host-side.
  - Edge MLP entirely in bf16 matmuls (1 cycle/row):
      X_pre = Wi.T @ node_T[:,i(bcast)] + Wj.T @ node_T[:,j(bcast)] (PSUM)
      X  = relu(X_pre + b1)   ScalarE evacuation -> bf16 SBUF
      h2 = relu(W2.T X + b2)  evacuations mostly ScalarE, 1-in-8 VectorE
      F  = W3blk.T h2         one-hot block weights accumulate (8,512)
  - F(8,512) -> F(64,64) via SBUF->SBUF DMA (identical linearized element
    order), F^T on TensorE, then
      out = (relu(G + F + F^T) + I) / row-sum
    with G = 0.25*(ap+ap^T) precomputed host-side and the 0.25
    symmetrization factor folded into W3.

Harness notes: walrus in this container accepts a single semaphore wait
per instruction, so a BIR-level pass splits Tile's multi-wait
instructions onto EventSemaphore carriers; the Tile kernel-tail drain
gets the same treatment at build time.
"""
import sys

sys.path.insert(0, '/opt/trn_rl_repo')

import numpy as np

B, N, H, T = 64, 64, 128, 128
NCORES = 8
B_LOC = B // NCORES      # 8 batches per core
PAIRS = B_LOC // 2       # 4 batch pairs per core
NCH = N // 8             # 8 i-chunks per batch (8 i x 64 j = 512 wide)

_CACHE = {}


def _install_wait_splitter():
    """walrus's per-instruction sync structs hold a single semaphore wait;
    Tile can emit several on one instruction. Split extras onto preceding
    single-wait Drain instructions at the BIR-JSON level."""
    if _CACHE.get('wait_splitter'):
        return
    import json

    import concourse.bass2jax as bass2jax

    orig = bass2jax.compile_bir_kernel

    def split_waits_in_bir(bir_bytes):
        d = json.loads(bir_bytes)
        n_new = [0]
        for fn in d.get("functions", []):
            for bb in fn.get("blocks", []):
                out = []
                for ins in bb.get("instructions", []):
                    si = ins.get("sync_info") or {}
                    waits = si.get("on_wait") or []
                    if len(waits) > 1:
                        for w in waits[:-1]:
                            n_new[0] += 1
                            out.append({
                                "engine": ins["engine"],
                                "ins": [],
                                "name": f"IWS-{n_new[0]}",
                                "opcode": "EventSemaphore",
                                "outs": [],
                                "sync_info": {"on_update": [], "on_wait": [w]},
                            })
                        si["on_wait"] = [waits[-1]]
                    out.append(ins)
                bb["instructions"] = out
        return json.dumps(d).encode()

    def wrapper(ant_bir_str, *a, **kw):
        return orig(split_waits_in_bir(ant_bir_str), *a, **kw)

    bass2jax.compile_bir_kernel = wrapper
    _CACHE['wait_splitter'] = True


def _split_drain_tile_context(tile_mod, mybir, nc):
    """TileContext whose kernel-tail drain splits its semaphore waits across
    sequential Drain instructions (walrus CTRL insts accept one wait)."""
    from concourse.tile import ScopedClock

    class SplitDrainTileContext(tile_mod.TileContext):
        def _drain_and_barrier(self, tick_clock, wait_clock):
            drain_inst = self.nc.sync.drain()
            wait_clock.add_sem_waits(
                drain_inst.ins, ScopedClock({None: tick_clock.global_clock})
            )
            waits = list(drain_inst.ins.sync_info.on_wait)
            if len(waits) > 1:
                drain_inst.ins.sync_info = mybir.SyncInfo(
                    on_wait=waits[:1],
                    on_update=list(drain_inst.ins.sync_info.on_update),
                )
                for i in range(1, len(waits)):
                    extra = self.nc.sync.drain()
                    extra.ins.sync_info = mybir.SyncInfo(
                        on_wait=waits[i : i + 1], on_update=[]
                    )
            self.nc.all_engine_barrier()
            assert self.sems is not None
            popped = self.nc._tile_sem_poison_stack.pop()
            assert popped is self._sem_poison
            self.nc.clear_and_free_semaphores(list(self.sems.allocated().values()))
            self.nc.all_engine_barrier()

    return SplitDrainTileContext(nc)


def build_nc():
    import concourse.bass as bass
    import concourse.tile as tile
    from concourse import mybir
    from contextlib import ExitStack

    f32 = mybir.dt.float32
    bf16 = mybir.dt.bfloat16
    AF = mybir.ActivationFunctionType
    ALU = mybir.AluOpType
    AX = mybir.AxisListType

    nc = bass.Bass()
    tf = nc.declare_dram_parameter("tf", [B_LOC, N, H, T], f32, isOutput=False)
    Wi = nc.declare_dram_parameter("Wi", [H, H], bf16, isOutput=False)
    Wj = nc.declare_dram_parameter("Wj", [H, H], bf16, isOutput=False)
    b1c = nc.declare_dram_parameter("b1c", [H, 1], f32, isOutput=False)
    W2 = nc.declare_dram_parameter("W2", [H, H // 2], bf16, isOutput=False)
    b2c = nc.declare_dram_parameter("b2c", [H // 2, 1], f32, isOutput=False)
    W3blk = nc.declare_dram_parameter("W3blk", [H // 2, 8 * NCH], bf16, isOutput=False)
    b3c = nc.declare_dram_parameter("b3c", [8, 1], f32, isOutput=False)
    G = nc.declare_dram_parameter("G", [N, N], f32, isOutput=False)
    I64 = nc.declare_dram_parameter("I64", [N, N], f32, isOutput=False)
    I128 = nc.declare_dram_parameter("I128", [H, H], f32, isOutput=False)
    out_ext = nc.declare_dram_parameter("out", [B_LOC, N, N], f32, isOutput=True)

    MONO = _CACHE.get('cfg_mono', False)
    NOBIAS = _CACHE.get('cfg_nobias', False)

    with _split_drain_tile_context(tile, mybir, nc) as tc, ExitStack() as ctx:
        consts = ctx.enter_context(tc.tile_pool(name="consts", bufs=1))
        tf_pool = ctx.enter_context(
            tc.tile_pool(name="tf", bufs=2 if MONO else 4))
        fold_pool = ctx.enter_context(tc.tile_pool(name="fold", bufs=2))
        red_pool = ctx.enter_context(tc.tile_pool(name="red", bufs=2))
        x_pool = ctx.enter_context(tc.tile_pool(name="x", bufs=4))
        h2_pool = ctx.enter_context(tc.tile_pool(name="h2", bufs=4))
        ff_pool = ctx.enter_context(tc.tile_pool(name="ff", bufs=2))
        ep_pool = ctx.enter_context(tc.tile_pool(name="ep", bufs=2))
        ps_x = ctx.enter_context(tc.tile_pool(name="ps_x", bufs=2, space="PSUM"))
        ps_h2 = ctx.enter_context(tc.tile_pool(name="ps_h2", bufs=2, space="PSUM"))
        ps_w3 = ctx.enter_context(tc.tile_pool(name="ps_w3", bufs=2, space="PSUM"))
        ps_t = ctx.enter_context(tc.tile_pool(name="ps_t", bufs=1, space="PSUM"))
        ps_ft = ctx.enter_context(tc.tile_pool(name="ps_ft", bufs=1, space="PSUM"))
        if True:
            wi_sb = consts.tile([H, H], bf16)
            nc.sync.dma_start(wi_sb[:], Wi[:])
            wj_sb = consts.tile([H, H], bf16)
            nc.sync.dma_start(wj_sb[:], Wj[:])
            w2_sb = consts.tile([H, H // 2], bf16)
            nc.sync.dma_start(w2_sb[:], W2[:])
            w3_sb = consts.tile([H // 2, 8 * NCH], bf16)
            nc.sync.dma_start(w3_sb[:], W3blk[:])
            b1_sb = consts.tile([H, 1], f32)
            nc.sync.dma_start(b1_sb[:], b1c[:])
            b2_sb = consts.tile([H // 2, 1], f32)
            nc.sync.dma_start(b2_sb[:], b2c[:])
            b3_sb = consts.tile([8, 1], f32)
            nc.sync.dma_start(b3_sb[:], b3c[:])
            g_sb = consts.tile([N, N], f32)
            nc.sync.dma_start(g_sb[:], G[:])
            i64_sb = consts.tile([N, N], f32)
            nc.sync.dma_start(i64_sb[:], I64[:])
            i128_sb = consts.tile([H, H], f32)
            nc.sync.dma_start(i128_sb[:], I128[:])

            HQ = H // 2

            def load_pair(c):
                if MONO:
                    # one monolithic 8MB transfer: single-queue DMA reaches
                    # ~425 GB/s only for large transfers (4MB: ~334)
                    tft = tf_pool.tile([128, H, T], f32, name=f"tft{c}",
                                       tag="tft")
                    nc.sync.dma_start(tft[:], tf[2 * c : 2 * c + 2, :, :, :])
                    return [tft[:, 0:HQ, :], tft[:, HQ:H, :]]
                halves = []
                for hh in range(2):
                    tft = tf_pool.tile([128, HQ, T], f32, name=f"tft{c}_{hh}",
                                       tag="tft")
                    # For the first two pairs, the odd halves ride the ACT
                    # HWDGE ring (its trigger sits at the head of ACT's
                    # still-idle queue) -> both rings run concurrently during
                    # the pipeline-fill window, ~2x load bandwidth.
                    eng = nc.scalar if (c < 2 and hh == 1) else nc.sync
                    eng.dma_start(
                        tft[:],
                        tf[2 * c : 2 * c + 2, :, hh * HQ : (hh + 1) * HQ, :])
                    halves.append(tft[:])
                return halves

            def load_pair0_quarters():
                # Pair 0 in 2MB quarters across both rings with a dedicated
                # pool tag: the first fold starts right after the first
                # quarter lands, and later pairs' half-tiles don't contend
                # for these slots.
                qs = []
                HF = H // 4
                for q in range(4):
                    tft = tf_pool.tile([128, HF, T], f32, name=f"tfq{q}",
                                       tag="tft")
                    eng = nc.sync if q % 2 == 0 else nc.scalar
                    eng.dma_start(
                        tft[:], tf[0:2, :, q * HF : (q + 1) * HF, :])
                    qs.append(tft[:])
                return qs

            def emit_folds(c, parts):
                # Sum over T: R[p=(b_lo,n), h] = sum_t tf[2c+b_lo, n, h, t].
                # Emitted BEFORE the previous pair's MLP so these sit ahead
                # of the wait-gated epilogue ops in the DVE FIFO and start
                # the moment their data lands.
                r_sb = red_pool.tile([128, H], f32, tag="r", name=f"r{c}")
                npc = len(parts)
                HP = H // npc
                for hh in range(npc):
                    th = parts[hh]
                    f1 = fold_pool.tile([128, HP, 64], bf16, tag="f1",
                                        name=f"f1_{c}_{hh}")
                    nc.vector.tensor_tensor(
                        f1[:], th[:, :, 0:64], th[:, :, 64:128], op=ALU.add)
                    f2 = fold_pool.tile([128, HP, 32], bf16, tag="f2",
                                        name=f"f2_{c}_{hh}")
                    nc.vector.tensor_tensor(
                        f2[:], f1[:, :, 0:32], f1[:, :, 32:64], op=ALU.add)
                    f3 = fold_pool.tile([128, HP, 16], bf16, tag="f3",
                                        name=f"f3_{c}_{hh}")
                    nc.vector.tensor_tensor(
                        f3[:], f2[:, :, 0:16], f2[:, :, 16:32], op=ALU.add)
                    nc.vector.reduce_sum(
                        r_sb[:, hh * HP : (hh + 1) * HP], f3[:], axis=AX.X)
                return r_sb

            pending = load_pair0_quarters()
            for c in range(PAIRS):
                parts = pending
                if c + 1 < PAIRS:
                    pending = load_pair(c + 1)
                r_sb = emit_folds(c, parts)
                # node_T[h, (b_lo, n)] via TensorE transpose (f32 in, bf16 out)
                rt_ps = ps_t.tile([128, 128], f32, tag="rt")
                nc.tensor.transpose(rt_ps[:], r_sb[:], i128_sb[:])
                rt_sb = red_pool.tile([128, 128], bf16, tag="rt_sb")
                nc.scalar.activation(rt_sb[:], rt_ps[:], AF.Copy)

                for b_lo in range(2):
                    b = 2 * c + b_lo
                    nodeb = rt_sb[:, 64 * b_lo : 64 * b_lo + 64]
                    w3_ps = ps_w3.tile([8, 512], f32, tag="w3")

                    def h2_stage(chunk, h2_ps):
                        # h2 evac + W3, emitted one chunk late: when this
                        # reaches ACT's FIFO head its W2 matmul finished a
                        # full stage ago -> no cross-engine ping-pong stall.
                        h2_sb = h2_pool.tile([64, 512], bf16, tag="h2",
                                             name=f"h2_{b}_{chunk}")
                        nc.scalar.activation(
                            h2_sb[:], h2_ps[:], AF.Relu,
                            bias=0.0 if NOBIAS else b2_sb[:])
                        nc.tensor.matmul(
                            w3_ps[:],
                            w3_sb[:, 8 * chunk : 8 * chunk + 8],
                            h2_sb[:],
                            start=(chunk == 0), stop=(chunk == NCH - 1))

                    prev = None
                    for chunk in range(NCH):
                        x_ps = ps_x.tile([128, 512], f32, tag="xps")
                        rhs_i = (
                            nodeb[:, 8 * chunk : 8 * chunk + 8]
                            .unsqueeze(2)
                            .broadcast_to((128, 8, 64)))
                        rhs_j = nodeb.unsqueeze(1).broadcast_to((128, 8, 64))
                        nc.tensor.matmul(
                            x_ps[:], wi_sb[:], rhs_i, start=True, stop=False)
                        nc.tensor.matmul(
                            x_ps[:], wj_sb[:], rhs_j, start=False, stop=True)
                        x_sb = x_pool.tile([128, 512], bf16, tag="x")
                        nc.scalar.activation(
                            x_sb[:], x_ps[:], AF.Relu,
                            bias=0.0 if NOBIAS else b1_sb[:])
                        h2_ps = ps_h2.tile([64, 512], f32, tag="h2ps")
                        nc.tensor.matmul(
                            h2_ps[:], w2_sb[:], x_sb[:], start=True, stop=True)
                        if prev is not None:
                            h2_stage(*prev)
                        prev = (chunk, h2_ps)
                    h2_stage(*prev)
                    # F_flat -> F (64, 64): same linearized element order
                    ff_sb = ff_pool.tile([8, 512], f32, tag="ff")
                    if NOBIAS:
                        nc.scalar.activation(ff_sb[:], w3_ps[:], AF.Copy)
                    else:
                        nc.scalar.activation(ff_sb[:], w3_ps[:], AF.Identity,
                                             bias=b3_sb[:])
                    f_sb = ep_pool.tile([N, N], f32, tag="f")
                    nc.sync.dma_start(f_sb[:], ff_sb[:])
                    ft_ps = ps_ft.tile([N, N], f32, tag="ft")
                    nc.tensor.transpose(ft_ps[:], f_sb[:], i64_sb[:, :64])
                    # out = (relu(G + F + F^T) + I) / (rowsum + 1e-8)
                    t1 = ep_pool.tile([N, N], f32, tag="t1")
                    nc.vector.tensor_tensor(t1[:], f_sb[:], ft_ps[:], op=ALU.add)
                    t2 = ep_pool.tile([N, N], f32, tag="t2")
                    nc.vector.tensor_tensor(t2[:], t1[:], g_sb[:], op=ALU.add)
                    sp = ep_pool.tile([N, N], f32, tag="sp")
                    nc.vector.tensor_scalar(
                        sp[:], t2[:], scalar1=0.0, scalar2=None, op0=ALU.max)
                    spi = ep_pool.tile([N, N], f32, tag="spi")
                    nc.vector.tensor_tensor(spi[:], sp[:], i64_sb[:], op=ALU.add)
                    rs = ep_pool.tile([N, 1], f32, tag="rs")
                    nc.vector.reduce_sum(rs[:], spi[:], axis=AX.X)
                    rb = ep_pool.tile([N, 1], f32, tag="rb")
                    nc.vector.tensor_scalar(
                        rb[:], rs[:], scalar1=1e-8, scalar2=None, op0=ALU.add)
                    rec = ep_pool.tile([N, 1], f32, tag="rec")
                    nc.vector.reciprocal(rec[:], rb[:])
                    o_sb = ep_pool.tile([N, N], f32, tag="o")
                    nc.vector.tensor_scalar(
                        o_sb[:], spi[:], scalar1=rec[:], scalar2=None,
                        op0=ALU.mult)
                    nc.sync.dma_start(out_ext[b], o_sb[:])
    return nc


def _get_nc():
    key = ('nc', _CACHE.get('cfg_mono', False), _CACHE.get('cfg_nobias', False))
    if key not in _CACHE:
        _CACHE[key] = build_nc()
    return _CACHE[key]


def kernel(**inputs):
    import ml_dtypes

    from concourse.bass_utils import run_bass_kernel_spmd

    _install_wait_splitter()

    tf = np.asarray(inputs["temporal_features"], dtype=np.float32)
    ap = np.asarray(inputs["adj_param"], dtype=np.float32)
    W1 = np.asarray(inputs["W1"], dtype=np.float32)
    b1 = np.asarray(inputs["b1"], dtype=np.float32)
    W2 = np.asarray(inputs["W2"], dtype=np.float32)
    b2 = np.asarray(inputs["b2"], dtype=np.float32)
    W3 = np.asarray(inputs["W3"], dtype=np.float32)
    b3 = np.asarray(inputs["b3"], dtype=np.float32)

    bf = ml_dtypes.bfloat16
    Wi = np.ascontiguousarray((W1[:H] / T).astype(bf))
    Wj = np.ascontiguousarray((W1[H:] / T).astype(bf))
    b1c = b1.reshape(H, 1)
    b2c = b2.reshape(H // 2, 1)
    # Per chunk, an (H//2, 8) one-hot-column weight routing the chunk's
    # scalar output to PSUM partition `chunk` (0.25 sym factor folded in).
    W3blk = np.zeros((H // 2, NCH, 8), np.float32)
    for chunk in range(NCH):
        W3blk[:, chunk, chunk] = 0.25 * W3[:, 0]
    W3blk = np.ascontiguousarray(W3blk.reshape(H // 2, 8 * NCH).astype(bf))
    b3c = np.full((8, 1), 0.25 * float(b3[0]), np.float32)
    G = np.ascontiguousarray(0.25 * (ap + ap.T))
    I64np = np.eye(N, dtype=np.float32)
    I128np = np.eye(H, dtype=np.float32)

    shared = {
        "Wi": Wi, "Wj": Wj, "b1c": b1c, "W2": np.ascontiguousarray(W2.astype(bf)),
        "b2c": b2c, "W3blk": W3blk, "b3c": b3c, "G": G, "I64": I64np,
        "I128": I128np,
    }
    in_maps = [
        {"tf": np.ascontiguousarray(tf[i * B_LOC : (i + 1) * B_LOC]), **shared}
        for i in range(NCORES)
    ]

    _CACHE['cfg_nobias'] = bool(
        not b1.any() and not b2.any() and not b3.any())
    nc = _get_nc()
    res = run_bass_kernel_spmd(nc, in_maps, core_ids=list(range(NCORES)),
                               **_CACHE.get('run_kwargs', {}))
    _CACHE['last_result'] = res
    out = np.concatenate([res.results[i]["out"] for i in range(NCORES)], axis=0)
    return np.ascontiguousarray(out.astype(np.float32))



# revision 22
# speedup vs baseline: 1.2411x; 1.0098x over previous
"""AdaptiveGraphLearning forward on 8 Trainium2 NeuronCores.

Data-parallel over batch B=64: each core processes 8 batches; the (N,N)
adjacency parameter and tiny edge-MLP weights are replicated (the forward
pass needs no collectives).

Per-core dataflow (8 batches as 4 pairs):
  - HWDGE-DMA two batches of temporal_features per transfer as
    (128, 8192) f32 tiles: partition p=(b_lo,n), free=(h,t) -- 32-64KB
    contiguous DRAM per partition for near-peak HBM bandwidth. Pair 0 is
    split into 2MB quarters spread over both HWDGE rings (SP + ACT) to
    shorten the pipeline-fill latency; later pairs ride the SP ring as
    4MB halves while ACT is busy with evacuations.
  - Sum over t: one f32->bf16 VectorE fold, then bf16 folds in 2x mode
    (t:128->64->32->16) and a short 1x reduce -> R (128=(b_lo,n), 128=h).
  - TensorE transpose -> node_T bf16 (h on partitions). 1/T folded into
    the W1 halves host-side.
  - Edge MLP entirely in bf16 matmuls (1 cycle/row):
      X_pre = Wi.T @ node_T[:,i(bcast)] + Wj.T @ node_T[:,j(bcast)] (PSUM)
      X  = relu(X_pre + b1)   ScalarE evacuation -> bf16 SBUF
      h2 = relu(W2.T X + b2)  evacuations mostly ScalarE, 1-in-8 VectorE
      F  = W3blk.T h2         one-hot block weights accumulate (8,512)
  - F(8,512) -> F(64,64) via SBUF->SBUF DMA (identical linearized element
    order), F^T on TensorE, then
      out = (relu(G + F + F^T) + I) / row-sum
    with G = 0.25*(ap+ap^T) precomputed host-side and the 0.25
    symmetrization factor folded into W3.

Harness notes: walrus in this container accepts a single semaphore wait
per instruction, so a BIR-level pass splits Tile's multi-wait
instructions onto EventSemaphore carriers; the Tile kernel-tail drain
gets the same treatment at build time.
"""
import sys

sys.path.insert(0, '/opt/trn_rl_repo')

import numpy as np

B, N, H, T = 64, 64, 128, 128
NCORES = 8
B_LOC = B // NCORES      # 8 batches per core
PAIRS = B_LOC // 2       # 4 batch pairs per core
NCH = N // 8             # 8 i-chunks per batch (8 i x 64 j = 512 wide)

_CACHE = {}


def _install_wait_splitter():
    """walrus's per-instruction sync structs hold a single semaphore wait;
    Tile can emit several on one instruction. Split extras onto preceding
    single-wait Drain instructions at the BIR-JSON level."""
    if _CACHE.get('wait_splitter'):
        return
    import json

    import concourse.bass2jax as bass2jax

    orig = bass2jax.compile_bir_kernel

    def split_waits_in_bir(bir_bytes):
        d = json.loads(bir_bytes)
        n_new = [0]
        for fn in d.get("functions", []):
            for bb in fn.get("blocks", []):
                out = []
                for ins in bb.get("instructions", []):
                    si = ins.get("sync_info") or {}
                    waits = si.get("on_wait") or []
                    if len(waits) > 1:
                        for w in waits[:-1]:
                            n_new[0] += 1
                            out.append({
                                "engine": ins["engine"],
                                "ins": [],
                                "name": f"IWS-{n_new[0]}",
                                "opcode": "EventSemaphore",
                                "outs": [],
                                "sync_info": {"on_update": [], "on_wait": [w]},
                            })
                        si["on_wait"] = [waits[-1]]
                    out.append(ins)
                bb["instructions"] = out
        return json.dumps(d).encode()

    def wrapper(ant_bir_str, *a, **kw):
        return orig(split_waits_in_bir(ant_bir_str), *a, **kw)

    bass2jax.compile_bir_kernel = wrapper
    _CACHE['wait_splitter'] = True


def _split_drain_tile_context(tile_mod, mybir, nc):
    """TileContext whose kernel-tail drain splits its semaphore waits across
    sequential Drain instructions (walrus CTRL insts accept one wait)."""
    from concourse.tile import ScopedClock

    class SplitDrainTileContext(tile_mod.TileContext):
        def _drain_and_barrier(self, tick_clock, wait_clock):
            drain_inst = self.nc.sync.drain()
            wait_clock.add_sem_waits(
                drain_inst.ins, ScopedClock({None: tick_clock.global_clock})
            )
            waits = list(drain_inst.ins.sync_info.on_wait)
            if len(waits) > 1:
                drain_inst.ins.sync_info = mybir.SyncInfo(
                    on_wait=waits[:1],
                    on_update=list(drain_inst.ins.sync_info.on_update),
                )
                for i in range(1, len(waits)):
                    extra = self.nc.sync.drain()
                    extra.ins.sync_info = mybir.SyncInfo(
                        on_wait=waits[i : i + 1], on_update=[]
                    )
            self.nc.all_engine_barrier()
            assert self.sems is not None
            popped = self.nc._tile_sem_poison_stack.pop()
            assert popped is self._sem_poison
            self.nc.clear_and_free_semaphores(list(self.sems.allocated().values()))
            self.nc.all_engine_barrier()

    return SplitDrainTileContext(nc)


def build_nc():
    import concourse.bass as bass
    import concourse.tile as tile
    from concourse import mybir
    from contextlib import ExitStack

    f32 = mybir.dt.float32
    bf16 = mybir.dt.bfloat16
    AF = mybir.ActivationFunctionType
    ALU = mybir.AluOpType
    AX = mybir.AxisListType

    nc = bass.Bass()
    tf = nc.declare_dram_parameter("tf", [B_LOC, N, H, T], f32, isOutput=False)
    Wi = nc.declare_dram_parameter("Wi", [H, H], bf16, isOutput=False)
    Wj = nc.declare_dram_parameter("Wj", [H, H], bf16, isOutput=False)
    b1c = nc.declare_dram_parameter("b1c", [H, 1], f32, isOutput=False)
    W2 = nc.declare_dram_parameter("W2", [H, H // 2], bf16, isOutput=False)
    b2c = nc.declare_dram_parameter("b2c", [H // 2, 1], f32, isOutput=False)
    W3blk = nc.declare_dram_parameter("W3blk", [H // 2, 8 * NCH], bf16, isOutput=False)
    b3c = nc.declare_dram_parameter("b3c", [8, 1], f32, isOutput=False)
    G = nc.declare_dram_parameter("G", [N, N], f32, isOutput=False)
    I64 = nc.declare_dram_parameter("I64", [N, N], f32, isOutput=False)
    I128 = nc.declare_dram_parameter("I128", [H, H], f32, isOutput=False)
    out_ext = nc.declare_dram_parameter("out", [B_LOC, N, N], f32, isOutput=True)

    MONO = _CACHE.get('cfg_mono', False)
    NOBIAS = _CACHE.get('cfg_nobias', False)

    with _split_drain_tile_context(tile, mybir, nc) as tc, ExitStack() as ctx:
        consts = ctx.enter_context(tc.tile_pool(name="consts", bufs=1))
        tf_pool = ctx.enter_context(
            tc.tile_pool(name="tf", bufs=2 if MONO else 4))
        fold_pool = ctx.enter_context(tc.tile_pool(name="fold", bufs=2))
        red_pool = ctx.enter_context(tc.tile_pool(name="red", bufs=2))
        x_pool = ctx.enter_context(tc.tile_pool(name="x", bufs=4))
        h2_pool = ctx.enter_context(tc.tile_pool(name="h2", bufs=4))
        ff_pool = ctx.enter_context(tc.tile_pool(name="ff", bufs=2))
        ep_pool = ctx.enter_context(tc.tile_pool(name="ep", bufs=2))
        ps_x = ctx.enter_context(tc.tile_pool(name="ps_x", bufs=2, space="PSUM"))
        ps_h2 = ctx.enter_context(tc.tile_pool(name="ps_h2", bufs=2, space="PSUM"))
        ps_w3 = ctx.enter_context(tc.tile_pool(name="ps_w3", bufs=2, space="PSUM"))
        ps_t = ctx.enter_context(tc.tile_pool(name="ps_t", bufs=1, space="PSUM"))
        ps_ft = ctx.enter_context(tc.tile_pool(name="ps_ft", bufs=1, space="PSUM"))
        if True:
            wi_sb = consts.tile([H, H], bf16)
            nc.scalar.dma_start(wi_sb[:], Wi[:])
            wj_sb = consts.tile([H, H], bf16)
            nc.scalar.dma_start(wj_sb[:], Wj[:])
            w2_sb = consts.tile([H, H // 2], bf16)
            nc.scalar.dma_start(w2_sb[:], W2[:])
            w3_sb = consts.tile([H // 2, 8 * NCH], bf16)
            nc.scalar.dma_start(w3_sb[:], W3blk[:])
            b1_sb = consts.tile([H, 1], f32)
            nc.scalar.dma_start(b1_sb[:], b1c[:])
            b2_sb = consts.tile([H // 2, 1], f32)
            nc.scalar.dma_start(b2_sb[:], b2c[:])
            b3_sb = consts.tile([8, 1], f32)
            nc.scalar.dma_start(b3_sb[:], b3c[:])
            g_sb = consts.tile([N, N], f32)
            nc.scalar.dma_start(g_sb[:], G[:])
            i64_sb = consts.tile([N, N], f32)
            nc.scalar.dma_start(i64_sb[:], I64[:])
            i128_sb = consts.tile([H, H], f32)
            nc.scalar.dma_start(i128_sb[:], I128[:])

            HQ = H // 2

            def load_pair(c):
                if MONO:
                    # one monolithic 8MB transfer: single-queue DMA reaches
                    # ~425 GB/s only for large transfers (4MB: ~334)
                    tft = tf_pool.tile([128, H, T], f32, name=f"tft{c}",
                                       tag="tft")
                    nc.sync.dma_start(tft[:], tf[2 * c : 2 * c + 2, :, :, :])
                    return [tft[:, 0:HQ, :], tft[:, HQ:H, :]]
                halves = []
                for hh in range(2):
                    tft = tf_pool.tile([128, HQ, T], f32, name=f"tft{c}_{hh}",
                                       tag="tft")
                    # For the first two pairs, the odd halves ride the ACT
                    # HWDGE ring (its trigger sits at the head of ACT's
                    # still-idle queue) -> both rings run concurrently during
                    # the pipeline-fill window, ~2x load bandwidth.
                    eng = nc.scalar if (c < 2 and hh == 1) else nc.sync
                    eng.dma_start(
                        tft[:],
                        tf[2 * c : 2 * c + 2, :, hh * HQ : (hh + 1) * HQ, :])
                    halves.append(tft[:])
                return halves

            def load_pair0_quarters():
                # Pair 0 in 2MB quarters across both rings with a dedicated
                # pool tag: the first fold starts right after the first
                # quarter lands, and later pairs' half-tiles don't contend
                # for these slots.
                qs = []
                HF = H // 4
                for q in range(4):
                    tft = tf_pool.tile([128, HF, T], f32, name=f"tfq{q}",
                                       tag="tft")
                    eng = nc.sync if q % 2 == 0 else nc.scalar
                    eng.dma_start(
                        tft[:], tf[0:2, :, q * HF : (q + 1) * HF, :])
                    qs.append(tft[:])
                return qs

            def emit_folds(c, parts):
                # Sum over T: R[p=(b_lo,n), h] = sum_t tf[2c+b_lo, n, h, t].
                # Emitted BEFORE the previous pair's MLP so these sit ahead
                # of the wait-gated epilogue ops in the DVE FIFO and start
                # the moment their data lands.
                r_sb = red_pool.tile([128, H], f32, tag="r", name=f"r{c}")
                npc = len(parts)
                HP = H // npc
                for hh in range(npc):
                    th = parts[hh]
                    f1 = fold_pool.tile([128, HP, 64], bf16, tag="f1",
                                        name=f"f1_{c}_{hh}")
                    nc.vector.tensor_tensor(
                        f1[:], th[:, :, 0:64], th[:, :, 64:128], op=ALU.add)
                    f2 = fold_pool.tile([128, HP, 32], bf16, tag="f2",
                                        name=f"f2_{c}_{hh}")
                    nc.vector.tensor_tensor(
                        f2[:], f1[:, :, 0:32], f1[:, :, 32:64], op=ALU.add)
                    f3 = fold_pool.tile([128, HP, 16], bf16, tag="f3",
                                        name=f"f3_{c}_{hh}")
                    nc.vector.tensor_tensor(
                        f3[:], f2[:, :, 0:16], f2[:, :, 16:32], op=ALU.add)
                    nc.vector.reduce_sum(
                        r_sb[:, hh * HP : (hh + 1) * HP], f3[:], axis=AX.X)
                return r_sb

            pending = load_pair0_quarters()
            for c in range(PAIRS):
                parts = pending
                if c + 1 < PAIRS:
                    pending = load_pair(c + 1)
                r_sb = emit_folds(c, parts)
                # node_T[h, (b_lo, n)] via TensorE transpose (f32 in, bf16 out)
                rt_ps = ps_t.tile([128, 128], f32, tag="rt")
                nc.tensor.transpose(rt_ps[:], r_sb[:], i128_sb[:])
                rt_sb = red_pool.tile([128, 128], bf16, tag="rt_sb")
                nc.scalar.activation(rt_sb[:], rt_ps[:], AF.Copy)

                for b_lo in range(2):
                    b = 2 * c + b_lo
                    nodeb = rt_sb[:, 64 * b_lo : 64 * b_lo + 64]
                    w3_ps = ps_w3.tile([8, 512], f32, tag="w3")

                    def h2_stage(chunk, h2_ps):
                        # h2 evac + W3, emitted one chunk late: when this
                        # reaches ACT's FIFO head its W2 matmul finished a
                        # full stage ago -> no cross-engine ping-pong stall.
                        h2_sb = h2_pool.tile([64, 512], bf16, tag="h2",
                                             name=f"h2_{b}_{chunk}")
                        nc.scalar.activation(
                            h2_sb[:], h2_ps[:], AF.Relu,
                            bias=0.0 if NOBIAS else b2_sb[:])
                        nc.tensor.matmul(
                            w3_ps[:],
                            w3_sb[:, 8 * chunk : 8 * chunk + 8],
                            h2_sb[:],
                            start=(chunk == 0), stop=(chunk == NCH - 1))

                    prev = None
                    for chunk in range(NCH):
                        x_ps = ps_x.tile([128, 512], f32, tag="xps")
                        rhs_i = (
                            nodeb[:, 8 * chunk : 8 * chunk + 8]
                            .unsqueeze(2)
                            .broadcast_to((128, 8, 64)))
                        rhs_j = nodeb.unsqueeze(1).broadcast_to((128, 8, 64))
                        nc.tensor.matmul(
                            x_ps[:], wi_sb[:], rhs_i, start=True, stop=False)
                        nc.tensor.matmul(
                            x_ps[:], wj_sb[:], rhs_j, start=False, stop=True)
                        x_sb = x_pool.tile([128, 512], bf16, tag="x")
                        nc.scalar.activation(
                            x_sb[:], x_ps[:], AF.Relu,
                            bias=0.0 if NOBIAS else b1_sb[:])
                        h2_ps = ps_h2.tile([64, 512], f32, tag="h2ps")
                        nc.tensor.matmul(
                            h2_ps[:], w2_sb[:], x_sb[:], start=True, stop=True)
                        if prev is not None:
                            h2_stage(*prev)
                        prev = (chunk, h2_ps)
                    h2_stage(*prev)
                    # F_flat -> F (64, 64): same linearized element order
                    ff_sb = ff_pool.tile([8, 512], f32, tag="ff")
                    if NOBIAS:
                        nc.scalar.activation(ff_sb[:], w3_ps[:], AF.Copy)
                    else:
                        nc.scalar.activation(ff_sb[:], w3_ps[:], AF.Identity,
                                             bias=b3_sb[:])
                    f_sb = ep_pool.tile([N, N], f32, tag="f")
                    nc.sync.dma_start(f_sb[:], ff_sb[:])
                    ft_ps = ps_ft.tile([N, N], f32, tag="ft")
                    nc.tensor.transpose(ft_ps[:], f_sb[:], i64_sb[:, :64])
                    # out = (relu(G + F + F^T) + I) / (rowsum + 1e-8)
                    t1 = ep_pool.tile([N, N], f32, tag="t1")
                    nc.vector.tensor_tensor(t1[:], f_sb[:], ft_ps[:], op=ALU.add)
                    t2 = ep_pool.tile([N, N], f32, tag="t2")
                    nc.vector.tensor_tensor(t2[:], t1[:], g_sb[:], op=ALU.add)
                    sp = ep_pool.tile([N, N], f32, tag="sp")
                    nc.vector.tensor_scalar(
                        sp[:], t2[:], scalar1=0.0, scalar2=None, op0=ALU.max)
                    spi = ep_pool.tile([N, N], f32, tag="spi")
                    nc.vector.tensor_tensor(spi[:], sp[:], i64_sb[:], op=ALU.add)
                    rs = ep_pool.tile([N, 1], f32, tag="rs")
                    nc.vector.reduce_sum(rs[:], spi[:], axis=AX.X)
                    rb = ep_pool.tile([N, 1], f32, tag="rb")
                    nc.vector.tensor_scalar(
                        rb[:], rs[:], scalar1=1e-8, scalar2=None, op0=ALU.add)
                    rec = ep_pool.tile([N, 1], f32, tag="rec")
                    nc.vector.reciprocal(rec[:], rb[:])
                    o_sb = ep_pool.tile([N, N], f32, tag="o")
                    nc.vector.tensor_scalar(
                        o_sb[:], spi[:], scalar1=rec[:], scalar2=None,
                        op0=ALU.mult)
                    nc.sync.dma_start(out_ext[b], o_sb[:])
    return nc


def _get_nc():
    key = ('nc', _CACHE.get('cfg_mono', False), _CACHE.get('cfg_nobias', False))
    if key not in _CACHE:
        _CACHE[key] = build_nc()
    return _CACHE[key]


def kernel(**inputs):
    import ml_dtypes

    from concourse.bass_utils import run_bass_kernel_spmd

    _install_wait_splitter()

    tf = np.asarray(inputs["temporal_features"], dtype=np.float32)
    ap = np.asarray(inputs["adj_param"], dtype=np.float32)
    W1 = np.asarray(inputs["W1"], dtype=np.float32)
    b1 = np.asarray(inputs["b1"], dtype=np.float32)
    W2 = np.asarray(inputs["W2"], dtype=np.float32)
    b2 = np.asarray(inputs["b2"], dtype=np.float32)
    W3 = np.asarray(inputs["W3"], dtype=np.float32)
    b3 = np.asarray(inputs["b3"], dtype=np.float32)

    bf = ml_dtypes.bfloat16
    Wi = np.ascontiguousarray((W1[:H] / T).astype(bf))
    Wj = np.ascontiguousarray((W1[H:] / T).astype(bf))
    b1c = b1.reshape(H, 1)
    b2c = b2.reshape(H // 2, 1)
    # Per chunk, an (H//2, 8) one-hot-column weight routing the chunk's
    # scalar output to PSUM partition `chunk` (0.25 sym factor folded in).
    W3blk = np.zeros((H // 2, NCH, 8), np.float32)
    for chunk in range(NCH):
        W3blk[:, chunk, chunk] = 0.25 * W3[:, 0]
    W3blk = np.ascontiguousarray(W3blk.reshape(H // 2, 8 * NCH).astype(bf))
    b3c = np.full((8, 1), 0.25 * float(b3[0]), np.float32)
    G = np.ascontiguousarray(0.25 * (ap + ap.T))
    I64np = np.eye(N, dtype=np.float32)
    I128np = np.eye(H, dtype=np.float32)

    shared = {
        "Wi": Wi, "Wj": Wj, "b1c": b1c, "W2": np.ascontiguousarray(W2.astype(bf)),
        "b2c": b2c, "W3blk": W3blk, "b3c": b3c, "G": G, "I64": I64np,
        "I128": I128np,
    }
    in_maps = [
        {"tf": np.ascontiguousarray(tf[i * B_LOC : (i + 1) * B_LOC]), **shared}
        for i in range(NCORES)
    ]

    _CACHE['cfg_nobias'] = bool(
        not b1.any() and not b2.any() and not b3.any())
    nc = _get_nc()
    res = run_bass_kernel_spmd(nc, in_maps, core_ids=list(range(NCORES)),
                               **_CACHE.get('run_kwargs', {}))
    _CACHE['last_result'] = res
    out = np.concatenate([res.results[i]["out"] for i in range(NCORES)], axis=0)
    return np.ascontiguousarray(out.astype(np.float32))

